# revision 1
# baseline (speedup 1.0000x reference)
"""Trainium2 kernel for nn_MultiHeadCrossAttention_28063316313030.

Math: with seq_len == 1, softmax over a size-1 axis is identically 1, so
attention(Q,K,V) == V and W_Q/W_K are dead code.  The whole module collapses to

    out = LN(x1 @ A) + LN(x2 @ A),   A = W_V.T @ W_fc.T   (1024 x 1024)

where LN is LayerNorm over the last dim with gamma/beta.

Distribution: pure data parallel over the batch dim across 8 NeuronCores.
Host precomputes A (tiny matmul) and stages x1/x2 transposed (C-major) so the
TensorE contraction dim lands on SBUF partitions with plain contiguous DMA.

Device per core (2048 rows per stream):
  for each 128-row tile, for each stream:
    z = xT_tile.T @ A        (f32r matmuls, 8 k-tiles x 2 PSUM banks of 512)
    bn_stats/bn_aggr -> mean/var;  r = 1/sqrt(var+eps) (ACT sqrt + DVE recip)
    n = z*r - mu*r           (ACT Identity with per-partition scale/bias)
  out_tile = n1 + n2 (DVE), optional gamma/beta affine, DMA out.
"""

import sys

sys.path.insert(0, "/opt/trn_rl_repo")

import numpy as np

B, C, OUT = 16384, 1024, 1024
EPS = 1e-5
NCORES = 8
R = B // NCORES  # rows per core per stream
P = 128
KT = C // P  # contraction tiles
BT = R // P  # row tiles per core
NH = OUT // 512  # psum bank halves per row tile

_cache = {}


def _build(use_affine: bool, mm_dtype_name: str):
    import concourse.bacc as bacc
    import concourse.mybir as mybir
    from concourse.tile import TileContext

    f32 = mybir.dt.float32
    mmdt = getattr(mybir.dt, mm_dtype_name)
    AF = mybir.ActivationFunctionType
    ALU = mybir.AluOpType

    nc = bacc.Bacc("TRN2", target_bir_lowering=False, debug=False, num_devices=NCORES)

    x1t = nc.declare_dram_parameter("x1t", [C, R], mmdt, isOutput=False)
    x2t = nc.declare_dram_parameter("x2t", [C, R], mmdt, isOutput=False)
    a_d = nc.declare_dram_parameter("a", [C, OUT], mmdt, isOutput=False)
    if use_affine:
        gam_d = nc.declare_dram_parameter("gamma", [OUT], f32, isOutput=False)
        bet2_d = nc.declare_dram_parameter("beta2", [OUT], f32, isOutput=False)
    y_d = nc.declare_dram_parameter("y", [R, OUT], f32, isOutput=True)

    with TileContext(nc) as tc:
        with (
            tc.tile_pool(name="singles", bufs=1) as singles,
            tc.tile_pool(name="xs", bufs=3) as xpool,
            tc.tile_pool(name="ns", bufs=3) as npool,
            tc.tile_pool(name="outs", bufs=3) as opool,
            tc.tile_pool(name="stats", bufs=4) as stats,
            tc.tile_pool(name="psum", bufs=2, space="PSUM") as psum,
        ):
            a_sb = singles.tile([P, KT, OUT], mmdt)
            nc.sync.dma_start(
                a_sb[:], a_d.rearrange("(ko ki) o -> ki ko o", ki=P)
            )
            eps_sb = singles.tile([P, 1], f32)
            nc.vector.memset(eps_sb, EPS)
            if use_affine:
                import concourse.bass as bass

                gam_sb = singles.tile([P, OUT], f32)
                nc.sync.dma_start(
                    gam_sb[:],
                    bass.AP(
                        tensor=gam_d.tensor,
                        offset=gam_d.offset,
                        ap=[[0, P], gam_d.ap[0]],
                    ),
                )
                bet2_sb = singles.tile([P, OUT], f32)
                nc.sync.dma_start(
                    bet2_sb[:],
                    bass.AP(
                        tensor=bet2_d.tensor,
                        offset=bet2_d.offset,
                        ap=[[0, P], bet2_d.ap[0]],
                    ),
                )

            x1r = x1t.rearrange("(ko ki) b -> ki ko b", ki=P)
            x2r = x2t.rearrange("(ko ki) b -> ki ko b", ki=P)

            for bt in range(BT):
                n_tiles = []
                for s, xr in enumerate((x1r, x2r)):
                    xt = xpool.tile([P, KT, P], mmdt, tag=f"xt{s}")
                    nc.sync.dma_start(xt[:], xr[:, :, bt * P : (bt + 1) * P])

                    zh = []
                    for h in range(NH):
                        ps = psum.tile([P, 512], f32, tag=f"ps{s}{h}")
                        for k in range(KT):
                            nc.tensor.matmul(
                                ps[:],
                                lhsT=xt[:, k, :],
                                rhs=a_sb[:, k, h * 512 : (h + 1) * 512],
                                start=(k == 0),
                                stop=(k == KT - 1),
                            )
                        zh.append(ps)

                    st = stats.tile([P, NH, 6], f32, tag=f"st{s}")
                    for h in range(NH):
                        nc.vector.bn_stats(st[:, h, :], zh[h][:])
                    mv = stats.tile([P, 2], f32, tag=f"mv{s}")
                    nc.vector.bn_aggr(mv[:], st[:])

                    # r = 1/sqrt(var + eps)
                    r_sb = stats.tile([P, 1], f32, tag=f"r{s}")
                    nc.scalar.activation(
                        r_sb[:], mv[:, 1:2], func=AF.Sqrt, bias=eps_sb[:], scale=1.0
                    )
                    nc.vector.reciprocal(r_sb[:], r_sb[:])
                    # nmr = -mean * r
                    nmr = stats.tile([P, 1], f32, tag=f"nmr{s}")
                    nc.vector.tensor_scalar(
                        nmr[:],
                        mv[:, 0:1],
                        scalar1=r_sb[:],
                        scalar2=-1.0,
                        op0=ALU.mult,
                        op1=ALU.mult,
                    )

                    ntile = npool.tile([P, OUT], f32, tag=f"n{s}")
                    for h in range(NH):
                        nc.scalar.activation(
                            ntile[:, h * 512 : (h + 1) * 512],
                            zh[h][:],
                            func=AF.Identity,
                            bias=nmr[:],
                            scale=r_sb[:],
                        )
                    n_tiles.append(ntile)

                out_t = opool.tile([P, OUT], f32, tag="out")
                nc.vector.tensor_tensor(
                    out_t[:], n_tiles[0][:], n_tiles[1][:], op=ALU.add
                )
                if use_affine:
                    nc.vector.tensor_tensor(
                        out_t[:], out_t[:], gam_sb[:], op=ALU.mult
                    )
                    nc.vector.tensor_tensor(
                        out_t[:], out_t[:], bet2_sb[:], op=ALU.add
                    )
                nc.sync.dma_start(y_d[bt * P : (bt + 1) * P, :], out_t[:])

    nc.finalize()
    return nc


def _get_nc(use_affine: bool, mm_dtype_name: str):
    key = (use_affine, mm_dtype_name)
    if key not in _cache:
        _cache[key] = _build(use_affine, mm_dtype_name)
    return _cache[key]


def kernel(x1, x2, W_Q, W_K, W_V, W_fc, gamma, beta, _trace=False,
           _mm_dtype="float32r"):
    from concourse.bass_utils import run_bass_kernel_spmd

    x1 = np.ascontiguousarray(np.asarray(x1, dtype=np.float32))
    x2 = np.ascontiguousarray(np.asarray(x2, dtype=np.float32))
    W_V = np.asarray(W_V, dtype=np.float32)
    W_fc = np.asarray(W_fc, dtype=np.float32)
    gamma = np.asarray(gamma, dtype=np.float32)
    beta = np.asarray(beta, dtype=np.float32)

    # A = W_V.T @ W_fc.T in float64 to keep the host collapse error negligible.
    A = (W_V.T.astype(np.float64) @ W_fc.T.astype(np.float64)).astype(np.float32)

    use_affine = not (np.all(gamma == 1.0) and np.all(beta == 0.0))

    x1t = np.ascontiguousarray(x1.T)  # [C, B]
    x2t = np.ascontiguousarray(x2.T)

    in_maps = []
    for r in range(NCORES):
        m = {
            "x1t": np.ascontiguousarray(x1t[:, r * R : (r + 1) * R]),
            "x2t": np.ascontiguousarray(x2t[:, r * R : (r + 1) * R]),
            "a": A,
        }
        if use_affine:
            m["gamma"] = gamma
            m["beta2"] = (2.0 * beta).astype(np.float32)
        in_maps.append(m)

    nc = _get_nc(use_affine, _mm_dtype)
    res = run_bass_kernel_spmd(nc, in_maps, list(range(NCORES)), trace=_trace)

    y = np.concatenate([res.results[r]["y"] for r in range(NCORES)], axis=0)
    out = y.reshape(B, 1, OUT)
    if _trace:
        return out, res
    return out
